# revision 4
# baseline (speedup 1.0000x reference)
"""Trainium2 Bass kernel for nn_MultiHeadAttention_60971355734022.

Full inputs in, full output out. Sharding: 8 cores = 4 batches x 2 head-groups
(8 heads each). Each core computes its (batch, head-group) slice end-to-end:
  - on-chip PE transposes of q/k/v tiles feed f32r projections
  - qhT/khT produced in [dh, s] layout; vh in [s, p] layout with a ones
    column per head (gives softmax denominators for free in the PV matmul)
  - causal attention computed as scores^T = khT.T-blocks @ qhT (so softmax
    normalization is deferred: PV accumulates unnormalized out^T + rowsum)
  - normalize with DVE reciprocal + GPSIMD partition broadcast
  - final projection contracts c^T (already in [p, s] layout) with Wf-slice
Host combines: out[b] = core(2b) + core(2b+1) + bf.
"""
import sys

sys.path.insert(0, "/opt/trn_rl_repo")

import math

import numpy as np

import concourse.bacc as bacc
import concourse.bass as bass
import concourse.tile as tile
from concourse import mybir
from concourse.bass_utils import run_bass_kernel_spmd

F32 = mybir.dt.float32
F32R = mybir.dt.float32r

S = 2048          # sequence length per batch
D = 1024          # model dim
P = 512           # per-core projection cols (8 heads x 64)
NH = 8            # heads per core
DH = 64           # head dim
NKB = S // 128    # 16 k-blocks
NCHUNK = 4        # s-chunks of 512 in phase A
SCALE = 1.0 / math.sqrt(2048.0)  # reference scales by 1/sqrt(MAX_LEN)

EXP = mybir.ActivationFunctionType.Exp


def build_core_kernel():
    nc = bacc.Bacc()

    qin = nc.dram_tensor("qin", [S, D], F32, kind="ExternalInput")
    kin = nc.dram_tensor("kin", [S, D], F32, kind="ExternalInput")
    vin = nc.dram_tensor("vin", [S, D], F32, kind="ExternalInput")
    wq = nc.dram_tensor("wq", [D, P], F32R, kind="ExternalInput")
    wk = nc.dram_tensor("wk", [D, P], F32R, kind="ExternalInput")
    wv = nc.dram_tensor("wv", [D, P], F32R, kind="ExternalInput")
    wf = nc.dram_tensor("wf", [P, D], F32R, kind="ExternalInput")
    bqv = nc.dram_tensor("bqv", [P], F32, kind="ExternalInput")
    bkv = nc.dram_tensor("bkv", [P], F32, kind="ExternalInput")
    bvv = nc.dram_tensor("bvv", [1, P], F32, kind="ExternalInput")
    eye = nc.dram_tensor("eye", [128, 128], F32, kind="ExternalInput")
    tri = nc.dram_tensor("tri", [128, 128], F32, kind="ExternalInput")
    out = nc.dram_tensor("out", [S, D], F32, kind="ExternalOutput")

    with tile.TileContext(nc) as tc:
        with tc.tile_pool(name="persist", bufs=1) as pp, \
             tc.tile_pool(name="ctp", bufs=1) as ctp:
            # persistent intermediates
            qhT = [pp.tile([128, S], F32R, name=f"qhT{i}", tag=f"qhT{i}")
                   for i in range(4)]
            khT = [pp.tile([128, S], F32R, name=f"khT{i}", tag=f"khT{i}")
                   for i in range(4)]
            vhh = pp.tile([128, NKB, NH, DH + 1], F32R, name="vhh", tag="vhh")
            cT = [ctp.tile([128, S], F32R, name=f"cT{i}", tag=f"cT{i}")
                  for i in range(4)]
            eyet = pp.tile([128, 128], F32, name="eyet", tag="eyet")
            trit = pp.tile([128, 128], F32, name="trit", tag="trit")
            bq_sb = pp.tile([128, 4], F32, name="bq_sb", tag="bq_sb")
            bk_sb = pp.tile([128, 4], F32, name="bk_sb", tag="bk_sb")
            bv_bc = pp.tile([128, P], F32, name="bv_bc", tag="bv_bc")
            nc.sync.dma_start(out=eyet, in_=eye[:, :])
            nc.sync.dma_start(out=trit, in_=tri[:, :])
            nc.sync.dma_start(out=bq_sb, in_=bqv.rearrange("(pb p) -> p pb", p=128))
            nc.sync.dma_start(out=bk_sb, in_=bkv.rearrange("(pb p) -> p pb", p=128))
            bv_row = pp.tile([1, P], F32, name="bv_row", tag="bv_row")
            nc.sync.dma_start(out=bv_row, in_=bvv[:, :])
            nc.gpsimd.partition_broadcast(bv_bc, bv_row)
            nc.vector.memset(vhh[:, :, :, DH:DH + 1].bitcast(F32), 1.0)

            # ---------------- Phase A: transpose + projections ----------------
            with tc.tile_pool(name="xnat", bufs=6) as xnatp, \
                 tc.tile_pool(name="xtp", bufs=1) as xtp, \
                 tc.tile_pool(name="wtp", bufs=2) as wtp, \
                 tc.tile_pool(name="tps", bufs=4, space="PSUM") as tpsp, \
                 tc.tile_pool(name="pjs", bufs=3, space="PSUM") as pjsp:
                for sc in range(NCHUNK):
                    for which, xin, wdram in (("q", qin, wq), ("k", kin, wk),
                                              ("v", vin, wv)):
                        xnats = []
                        for sb in range(4):
                            r0 = sc * 512 + sb * 128
                            xnat = xnatp.tile([128, D], F32,
                                              name=f"xn_{which}{sc}_{sb}", tag="xn")
                            nc.gpsimd.dma_start(out=xnat, in_=xin[r0:r0 + 128, :])
                            xnats.append(xnat)
                        wt = wtp.tile([128, 8, P], F32R, name=f"wt_{which}{sc}",
                                      tag="wt")
                        nc.sync.dma_start(
                            out=wt, in_=wdram.rearrange("(db p) c -> p db c", p=128))
                        xt = xtp.tile([128, 8, 512], F32R,
                                      name=f"xt_{which}{sc}", tag="xt")
                        for db in range(8):
                            tp = tpsp.tile([128, 512], F32,
                                           name=f"tp{sc}{db}", tag="tp")
                            for sb in range(4):
                                nc.tensor.matmul(
                                    tp[:, 128 * sb:128 * sb + 128],
                                    xnats[sb][:, 128 * db:128 * db + 128],
                                    eyet, is_transpose=True,
                                    start=(sb == 0), stop=(sb == 3))
                            nc.vector.tensor_copy(xt[:, db, :], tp[:, :])
                        if which in ("q", "k"):
                            dst = qhT if which == "q" else khT
                            bias = bq_sb if which == "q" else bk_sb
                            for pb in range(4):
                                pj = pjsp.tile([128, 512], F32,
                                               name=f"pj_{which}{sc}{pb}", tag="pj")
                                for db in range(8):
                                    nc.tensor.matmul(
                                        pj[:, :],
                                        wt[:, db, 128 * pb:128 * pb + 128],
                                        xt[:, db, :],
                                        start=(db == 0), stop=(db == 7))
                                nc.vector.tensor_scalar_add(
                                    dst[pb][:, 512 * sc:512 * (sc + 1)], pj[:, :],
                                    bias[:, pb:pb + 1])
                        else:
                            for sb in range(4):
                                pj = pjsp.tile([128, 512], F32,
                                               name=f"pj_v{sc}{sb}", tag="pj")
                                for db in range(8):
                                    nc.tensor.matmul(
                                        pj[:, :],
                                        xt[:, db, 128 * sb:128 * sb + 128],
                                        wt[:, db, :],
                                        start=(db == 0), stop=(db == 7))
                                nc.vector.scalar_tensor_tensor(
                                    vhh[:, sc * 4 + sb, :, 0:DH],
                                    pj.rearrange("p (h d) -> p h d", h=NH),
                                    1.0,
                                    bv_bc.rearrange("p (h d) -> p h d", h=NH),
                                    mybir.AluOpType.mult,
                                    mybir.AluOpType.add)

            # ---------------- Phase B: causal attention ----------------
            with tc.tile_pool(name="scs", bufs=2, space="PSUM") as scsp, \
                 tc.tile_pool(name="ops", bufs=1, space="PSUM") as opsp, \
                 tc.tile_pool(name="ptp", bufs=4) as ptp, \
                 tc.tile_pool(name="nrm", bufs=4) as nrmp:
                for hp in range(4):
                    for ps in range(2):
                        qlo = 1024 * ps
                        qhi = qlo + 1024
                        opsum = [[opsp.tile([DH + 1, 512], F32,
                                            name=f"op{hp}{ps}{h}{qc}",
                                            tag=f"op{h}{qc}")
                                  for qc in range(2)] for h in range(2)]
                        nkb_p = qhi // 128
                        for kb in range(nkb_p):
                            span0 = max(qlo, 128 * kb)
                            o0 = span0 - qlo
                            for h in range(2):
                                sp = scsp.tile([128, 1024], F32,
                                               name=f"sp{hp}{ps}{kb}{h}", tag="sp")
                                lhs = khT[hp][64 * h:64 * h + 64,
                                              128 * kb:128 * kb + 128]
                                if o0 < 512:
                                    nc.tensor.matmul(
                                        sp[:, o0:512], lhs,
                                        qhT[hp][64 * h:64 * h + 64, span0:qlo + 512],
                                        start=True, stop=True,
                                        tile_position=(64 * h, 0))
                                    nc.tensor.matmul(
                                        sp[:, 512:1024], lhs,
                                        qhT[hp][64 * h:64 * h + 64, qlo + 512:qhi],
                                        start=True, stop=True,
                                        tile_position=(64 * h, 0))
                                else:
                                    nc.tensor.matmul(
                                        sp[:, o0:1024], lhs,
                                        qhT[hp][64 * h:64 * h + 64, span0:qhi],
                                        start=True, stop=True,
                                        tile_position=(64 * h, 0))
                                pt = ptp.tile([128, 1024], F32R,
                                              name=f"pt{hp}{ps}{kb}{h}", tag="pt")
                                nc.scalar.activation(pt[:, o0:1024], sp[:, o0:1024],
                                                     EXP, scale=SCALE)
                                if 128 * kb >= qlo:
                                    nc.gpsimd.affine_select(
                                        pt[:, o0:o0 + 128], pt[:, o0:o0 + 128],
                                        pattern=[[1, 128]],
                                        compare_op=mybir.AluOpType.is_ge,
                                        fill=0.0, base=0, channel_multiplier=-1)
                                for qc in range(2):
                                    lo = qlo + 512 * qc
                                    hi = lo + 512
                                    if 128 * kb >= hi:
                                        continue
                                    vstart = max(span0, lo)
                                    last_kb = hi // 128 - 1
                                    nc.tensor.matmul(
                                        opsum[h][qc][:, vstart - lo:512],
                                        vhh[:, kb, 2 * hp + h, :],
                                        pt[:, vstart - qlo:hi - qlo],
                                        start=(kb == 0), stop=(kb == last_kb))
                                    if kb == last_kb:
                                        rec = nrmp.tile([1, 512], F32,
                                                        name=f"rc{hp}{ps}{h}{qc}",
                                                        tag="rc")
                                        nc.vector.reciprocal(
                                            rec, opsum[h][qc][DH:DH + 1, :])
                                        rbc = nrmp.tile([64, 512], F32,
                                                        name=f"rb{hp}{ps}{h}{qc}",
                                                        tag="rb")
                                        nc.gpsimd.partition_broadcast(rbc, rec)
                                        nc.vector.tensor_mul(
                                            cT[hp][64 * h:64 * h + 64, lo:hi],
                                            opsum[h][qc][0:DH, :], rbc)

            # ---------------- Phase C: output projection ----------------
            with tc.tile_pool(name="wfp", bufs=1) as wfp, \
                 tc.tile_pool(name="fps", bufs=4, space="PSUM") as fpsp, \
                 tc.tile_pool(name="osg", bufs=4) as osgp:
                wft = wfp.tile([128, 4, D], F32R, name="wft", tag="wft")
                nc.sync.dma_start(
                    out=wft, in_=wf.rearrange("(hp p) c -> p hp c", p=128))
                for sb in range(16):
                    for dm in range(2):
                        fp = fpsp.tile([128, 512], F32,
                                       name=f"fp{sb}{dm}", tag="fp")
                        for hp in range(4):
                            nc.tensor.matmul(
                                fp[:, :],
                                cT[hp][:, 128 * sb:128 * sb + 128],
                                wft[:, hp, 512 * dm:512 * dm + 512],
                                start=(hp == 0), stop=(hp == 3))
                        osg = osgp.tile([128, 512], F32,
                                        name=f"os{sb}{dm}", tag="os")
                        nc.vector.tensor_copy(osg, fp[:, :])
                        nc.sync.dma_start(
                            out=out[128 * sb:128 * sb + 128,
                                    512 * dm:512 * dm + 512],
                            in_=osg)
    nc.finalize()
    return nc


_NC_CACHE = None


def _get_nc():
    global _NC_CACHE
    if _NC_CACHE is None:
        _NC_CACHE = build_core_kernel()
    return _NC_CACHE


def kernel(q, k, v, Wq, bq, Wk, bk, Wv, bv, Wf, bf, trace=False, tmpdir=None):
    q = np.asarray(q, np.float32)
    k = np.asarray(k, np.float32)
    v = np.asarray(v, np.float32)
    Wq = np.asarray(Wq, np.float32)
    Wk = np.asarray(Wk, np.float32)
    Wv = np.asarray(Wv, np.float32)
    Wf = np.asarray(Wf, np.float32)
    bq = np.asarray(bq, np.float32)
    bk = np.asarray(bk, np.float32)
    bv = np.asarray(bv, np.float32)
    bf = np.asarray(bf, np.float32)

    eye = np.eye(128, dtype=np.float32)
    tri = np.triu(np.ones((128, 128), np.float32))  # row=k, col=q; keep q>=k

    in_maps = []
    for c in range(8):
        b, g = c // 2, c % 2
        sl = slice(P * g, P * (g + 1))
        in_maps.append({
            "qin": np.ascontiguousarray(q[b]),
            "kin": np.ascontiguousarray(k[b]),
            "vin": np.ascontiguousarray(v[b]),
            "wq": np.ascontiguousarray(Wq[:, sl]),
            "wk": np.ascontiguousarray(Wk[:, sl]),
            "wv": np.ascontiguousarray(Wv[:, sl]),
            "wf": np.ascontiguousarray(Wf[sl, :]),
            "bqv": np.ascontiguousarray(bq[sl]),
            "bkv": np.ascontiguousarray(bk[sl]),
            "bvv": np.ascontiguousarray(bv[sl])[None, :],
            "eye": eye,
            "tri": tri,
        })

    nc = _get_nc()
    kw = {}
    if trace:
        kw = {"trace": True, "tmpdir": tmpdir}
    res = run_bass_kernel_spmd(nc, in_maps, core_ids=list(range(8)), **kw)

    outp = np.empty((4, S, D), np.float32)
    for b in range(4):
        outp[b] = res.results[2 * b]["out"] + res.results[2 * b + 1]["out"] + bf
    if trace:
        return outp, res
    return outp
